# revision 18
# baseline (speedup 1.0000x reference)
"""Causal multi-head attention on 8 Trainium2 NeuronCores.

Problem: x[2,4096,512], W_q/W_k/W_v/W_proj[512,512], b_proj[512]
  q,k,v = x @ W.T split into 8 heads of 64; causal softmax(q k^T / 8) v;
  out = attn @ W_proj.T + b_proj.

Sharding: 16 (batch, head) pairs over 8 cores -> each core gets one batch
and a pair of adjacent heads (128 of the 512 hidden dims).  The output
projection is computed per-core against the matching 128-row slice of
W_proj^T, giving a partial [4096, 512] output per core; the host sums the
4 partials per batch and adds the bias.

v2 design (bf16 compute, fp32 accumulate):
  xT   [512, 4096] bf16 (host-cast, host-transposed input slice)
  qT/kT [128, 4096] bf16: rows 0-63 head0, 64-127 head1
  scores per k-block: two row-tiled matmuls (h0 rows 0-63, h1 rows 64-127
  of the PE array, concurrent) -> one [128 k, 1024] PSUM tile (h0|h1)
  exp via ScalarE (scale=1/8 folded), causal mask multiply only on
  diagonal blocks; ex bf16.
  attnT accumulated as [65, 512] fp32 per head: rows 0-63 v^T ex, row 64
  softmax denominators (ones column appended to v).
  normalize: reciprocal of denominators + ones-outer-product broadcast
  matmul, fused into the PSUM->SBUF evacuation multiply.
"""

import numpy as np

B, S, D, H = 2, 4096, 512, 8
DH = 64
QCHUNK = 512
SCALE = 1.0 / np.sqrt(DH)

_CACHE = {}


def _build(s=S, normalize=True, repeats=1):
    from contextlib import ExitStack

    import concourse.mybir as mybir
    import concourse.tile as tile
    from concourse import bacc

    f32 = mybir.dt.float32
    f32r = mybir.dt.float32r
    bf16 = mybir.dt.bfloat16
    EXP = mybir.ActivationFunctionType.Exp
    GE = mybir.AluOpType.is_ge

    nqc = s // QCHUNK      # q chunks
    nkb_all = s // 128     # k blocks
    ndc = D // 128         # D chunks (contraction for projections)
    kb_per_chunk = QCHUNK // 128

    nc = bacc.Bacc("TRN2")
    xT_d = nc.dram_tensor("xT", [D, s], bf16, kind="ExternalInput")
    wqT_d = nc.dram_tensor("wqT", [D, 128], bf16, kind="ExternalInput")
    wkT_d = nc.dram_tensor("wkT", [D, 128], bf16, kind="ExternalInput")
    wvT_d = nc.dram_tensor("wvT", [D, 128], bf16, kind="ExternalInput")
    wpT_d = nc.dram_tensor("wpT", [128, D], bf16, kind="ExternalInput")
    ones_d = nc.dram_tensor("ones_in", [128, 64], bf16, kind="ExternalInput")
    out_d = nc.dram_tensor("out_p", [s, D], f32, kind="ExternalOutput")

    with ExitStack() as ctx:
        tc = ctx.enter_context(tile.TileContext(nc))
        consts = ctx.enter_context(tc.tile_pool(name="consts", bufs=1))
        big = ctx.enter_context(tc.tile_pool(name="big", bufs=1))
        expool = ctx.enter_context(tc.tile_pool(name="expool", bufs=4))
        denpool = ctx.enter_context(tc.tile_pool(name="denpool", bufs=2))
        outpool = ctx.enter_context(tc.tile_pool(name="outpool", bufs=2))
        mmps = ctx.enter_context(tc.tile_pool(name="mmps", bufs=2, space="PSUM"))
        scps = ctx.enter_context(tc.tile_pool(name="scps", bufs=2, space="PSUM"))
        accps = ctx.enter_context(tc.tile_pool(name="accps", bufs=1, space="PSUM"))

        # ---- persistent SBUF ----
        xT = [big.tile([128, s], bf16, name=f"xT{c}", tag=f"xT{c}") for c in range(ndc)]
        qT = big.tile([128, s], bf16, name="qT", tag="qT")
        kT = big.tile([128, s], bf16, name="kT", tag="kT")
        v65 = [big.tile([128, 65 * nkb_all], bf16, name=f"v65_{h}", tag=f"v65_{h}")
               for h in range(2)]
        attnT = big.tile([128, s], bf16, name="attnT", tag="attnT")
        wq = consts.tile([128, D], bf16, name="wq", tag="wq")
        wk = consts.tile([128, D], bf16, name="wk", tag="wk")
        wv = consts.tile([128, D], bf16, name="wv", tag="wv")
        wp = consts.tile([128, D], bf16, name="wp", tag="wp")
        masks = [consts.tile([128, 1024], bf16, name=f"mask{r}", tag=f"mask{r}")
                 for r in range(kb_per_chunk)]
        ones33 = consts.tile([33, 64], f32r, name="ones33", tag="ones33")

        for _rep in range(repeats):
            _emit_body(nc, tc, locals())

    nc.compile()
    return nc


def _emit_body(nc, tc, env):
    """One full pass of the kernel body (DMAs + all chunks)."""
    import concourse.mybir as mybir

    f32 = mybir.dt.float32
    f32r = mybir.dt.float32r
    bf16 = mybir.dt.bfloat16
    EXP = mybir.ActivationFunctionType.Exp
    GE = mybir.AluOpType.is_ge
    (s, nqc, nkb_all, ndc, kb_per_chunk, normalize) = (
        env["s"], env["nqc"], env["nkb_all"], env["ndc"],
        env["kb_per_chunk"], env["normalize"])
    (xT_d, wqT_d, wkT_d, wvT_d, wpT_d, ones_d, out_d) = (
        env["xT_d"], env["wqT_d"], env["wkT_d"], env["wvT_d"], env["wpT_d"],
        env["ones_d"], env["out_d"])
    (xT, qT, kT, v65, attnT, wq, wk, wv, wp, masks, ones33) = (
        env["xT"], env["qT"], env["kT"], env["v65"], env["attnT"], env["wq"],
        env["wk"], env["wv"], env["wp"], env["masks"], env["ones33"])
    (consts, big, expool, denpool, outpool, mmps, scps, accps) = (
        env["consts"], env["big"], env["expool"], env["denpool"],
        env["outpool"], env["mmps"], env["scps"], env["accps"])
    QCHUNK = 512

    # ---- input DMAs (x split so chunk 0 unblocks early) ----
    half = s // 2
    for lo, hi in ((0, QCHUNK), (QCHUNK, half), (half, s)):
        for c in range(ndc):
            nc.sync.dma_start(out=xT[c][:, lo:hi],
                              in_=xT_d[c * 128:(c + 1) * 128, lo:hi])
    for w_sb, w_d in ((wq, wqT_d), (wk, wkT_d), (wv, wvT_d)):
        nc.sync.dma_start(
            out=w_sb.rearrange("p (c j) -> p c j", j=128),
            in_=w_d.rearrange("(c p) j -> p c j", p=128))
    nc.sync.dma_start(out=wp, in_=wpT_d.ap())
    for h in range(2):
        ones_ap = v65[h].rearrange("p (k c) -> p k c", c=65)[:, :, 64]
        nc.sync.dma_start(out=ones_ap, in_=ones_d[:, 0:nkb_all])
    nc.gpsimd.memset(ones33.bitcast(f32), 1.0)

    # ---- causal masks: mask[r][p, f%512] = 1.0 where (f%512) >= p + 128*r ----
    mtmp = denpool.tile([128, 512], f32, name="mtmp", tag="mtmp")
    for r in range(kb_per_chunk):
        nc.gpsimd.memset(mtmp, 1.0)
        nc.gpsimd.affine_select(
            out=mtmp, in_=mtmp, compare_op=GE, fill=0.0,
            base=-128 * r, channel_multiplier=-1, pattern=[[1, 512]])
        nc.vector.tensor_copy(masks[r][:, 0:512], mtmp)
        nc.vector.tensor_copy(masks[r][:, 512:1024], mtmp)

    def phase1_units(qc):
        """Projection work for q-range qc as independently-emittable units
        (interleaved into the previous chunk's k-block loop so the PE fills
        its exp-wait stalls with them)."""
        qs = slice(qc * QCHUNK, (qc + 1) * QCHUNK)

        def proj_unit(w_sb, dst):
            ps = mmps.tile([128, QCHUNK], f32, name=f"proj_{qc}", tag="mm")
            for c in range(ndc):
                nc.tensor.matmul(ps,
                                 lhsT=w_sb[:, c * 128:(c + 1) * 128],
                                 rhs=xT[c][:, qs],
                                 start=(c == 0), stop=(c == ndc - 1))
            nc.vector.tensor_copy(dst[:, qs], ps)

        def v_unit(j):
            # natural-layout v rows: one [128,128] block (both heads)
            kb = qc * kb_per_chunk + j
            vp = mmps.tile([128, 128], f32, name=f"vp_{kb}", tag="mm")
            for c in range(ndc):
                nc.tensor.matmul(vp,
                                 lhsT=xT[c][:, kb * 128:(kb + 1) * 128],
                                 rhs=wv[:, c * 128:(c + 1) * 128],
                                 start=(c == 0), stop=(c == ndc - 1))
            for h in range(2):
                nc.vector.tensor_copy(v65[h][:, kb * 65:kb * 65 + 64],
                                      vp[:, h * 64:(h + 1) * 64])

        units = [lambda w=w, d=d: proj_unit(w, d)
                 for w, d in ((wq, qT), (wk, kT))]
        units += [lambda j=j: v_unit(j) for j in range(kb_per_chunk)]
        return units

    for u in phase1_units(0):
        u()
    for qc in range(nqc):
        qlo = qc * QCHUNK
        qs = slice(qlo, qlo + QCHUNK)

        # ---- phase 2: attention over k blocks, next chunk's projections
        #      interleaved ----
        nkb = (qc + 1) * kb_per_chunk
        units = phase1_units(qc + 1) if qc + 1 < nqc else []
        nunits, emitted_units = len(units), 0
        acc = [accps.tile([65, QCHUNK], f32, name=f"acc{h}_{qc}", tag=f"acc{h}")
               for h in range(2)]
        for kb in range(nkb):
            sc = scps.tile([128, 1024], f32, name=f"sc_{qc}_{kb}", tag="sc")
            for h in range(2):
                hsl = slice(h * 64, (h + 1) * 64)
                nc.tensor.matmul(
                    sc[:, h * 512:(h + 1) * 512],
                    lhsT=kT[hsl, kb * 128:(kb + 1) * 128],
                    rhs=qT[hsl, qs],
                    start=True, stop=True)
            ex = expool.tile([128, 1024], bf16, name=f"ex_{qc}_{kb}", tag="ex")
            nc.scalar.activation(ex, sc, EXP, scale=float(SCALE))
            if kb * 128 >= qlo:  # diagonal block: zero where k > q
                # gpsimd (otherwise idle) so DVE stays free for evacuations
                nc.gpsimd.tensor_mul(ex, ex, masks[kb - qc * kb_per_chunk])
            for h in range(2):
                nc.tensor.matmul(
                    acc[h],
                    lhsT=v65[h][:, kb * 65:(kb + 1) * 65],
                    rhs=ex[:, h * 512:(h + 1) * 512],
                    start=(kb == 0), stop=(kb == nkb - 1))
            # spread next-chunk projection units across this kb loop
            while units and emitted_units * nkb <= (kb + 1) * nunits:
                units.pop(0)()
                emitted_units += 1

        for u in units:
            u()

        # ---- phase 3: normalize + output projection for this q-chunk ----
        if normalize:
            # head h's denominator row sits at partition 32*h (32-aligned)
            dens = denpool.tile([33, QCHUNK], f32r, name=f"dens_{qc}",
                                tag="dens")
            for h in range(2):
                nc.vector.tensor_copy(dens[32 * h:32 * h + 1, :],
                                      acc[h][64:65, :])
            for h in range(2):
                hsl = slice(h * 64, (h + 1) * 64)
                # broadcast den across 64 partitions via ones outer-product,
                # then approx-reciprocal evacuates PSUM->SBUF
                bc = mmps.tile([64, QCHUNK], f32, name=f"bc{h}_{qc}", tag="mm")
                nc.tensor.matmul(bc, lhsT=ones33[32 * h:32 * h + 1, 0:64],
                                 rhs=dens[32 * h:32 * h + 1, :],
                                 start=True, stop=True)
                rbc = denpool.tile([64, QCHUNK], f32, name=f"rbc{h}_{qc}",
                                   tag=f"rbc{h}")
                nc.vector.reciprocal_approx_fast(out=rbc, in_=bc)
                nc.vector.tensor_mul(attnT[hsl, qs], acc[h][0:64, :], rbc)
        else:
            for h in range(2):
                nc.vector.tensor_copy(attnT[h * 64:(h + 1) * 64, qs],
                                      acc[h][0:64, :])

        ob = outpool.tile([128, kb_per_chunk * D], f32, name=f"ob_{qc}",
                          tag="ob")
        for j in range(kb_per_chunk):
            qb = qc * kb_per_chunk + j
            pp = mmps.tile([128, D], f32, name=f"pp_{qb}", tag="mm")
            nc.tensor.matmul(pp,
                             lhsT=attnT[:, qb * 128:(qb + 1) * 128],
                             rhs=wp, start=True, stop=True)
            nc.vector.tensor_copy(ob[:, j * D:(j + 1) * D], pp)
        nc.sync.dma_start(
            out=out_d.rearrange("(q j p) d -> q p j d", j=kb_per_chunk,
                                p=128)[qc],
            in_=ob.rearrange("p (j d) -> p j d", d=D))


def _in_maps(x, W_q, W_k, W_v, W_proj):
    import ml_dtypes
    bf16 = ml_dtypes.bfloat16
    maps = []
    for c in range(8):
        b, hp = c // 4, c % 4
        cols = slice(hp * 128, (hp + 1) * 128)
        maps.append({
            "xT": np.ascontiguousarray(x[b].T).astype(bf16),
            "wqT": np.ascontiguousarray(W_q.T[:, cols]).astype(bf16),
            "wkT": np.ascontiguousarray(W_k.T[:, cols]).astype(bf16),
            "wvT": np.ascontiguousarray(W_v.T[:, cols]).astype(bf16),
            "wpT": np.ascontiguousarray(W_proj[:, cols].T).astype(bf16),
            "ones_in": np.ones((128, 64), dtype=bf16),
        })
    return maps


def kernel(x, W_q, W_k, W_v, W_proj, b_proj, _trace=False):
    from concourse.bass_utils import run_bass_kernel_spmd

    x = np.asarray(x, dtype=np.float32)
    W_q = np.asarray(W_q, dtype=np.float32)
    W_k = np.asarray(W_k, dtype=np.float32)
    W_v = np.asarray(W_v, dtype=np.float32)
    W_proj = np.asarray(W_proj, dtype=np.float32)
    b_proj = np.asarray(b_proj, dtype=np.float32)

    if "nc" not in _CACHE:
        _CACHE["nc"] = _build()
    nc = _CACHE["nc"]

    res = run_bass_kernel_spmd(nc, _in_maps(x, W_q, W_k, W_v, W_proj),
                               core_ids=list(range(8)), trace=_trace)
    out = np.empty((B, S, D), dtype=np.float32)
    for b in range(B):
        acc = res.results[4 * b]["out_p"].astype(np.float32)
        for j in range(1, 4):
            acc = acc + res.results[4 * b + j]["out_p"]
        out[b] = acc + b_proj
    if _trace:
        _CACHE["last_trace"] = res
    return out


# revision 20
# speedup vs baseline: 1.8788x; 1.8788x over previous
"""Causal multi-head attention on 8 Trainium2 NeuronCores.

Problem: x[2,4096,512], W_q/W_k/W_v/W_proj[512,512], b_proj[512]
  q,k,v = x @ W.T split into 8 heads of 64; causal softmax(q k^T / 8) v;
  out = attn @ W_proj.T + b_proj.

Sharding: 16 (batch, head) pairs over 8 cores -> each core gets one batch
and a pair of adjacent heads (128 of the 512 hidden dims).  The output
projection is computed per-core against the matching 128-row slice of
W_proj^T, giving a partial [4096, 512] output per core; the host sums the
4 partials per batch and adds the bias.

v2 design (bf16 compute, fp32 accumulate):
  xT   [512, 4096] bf16 (host-cast, host-transposed input slice)
  qT/kT [128, 4096] bf16: rows 0-63 head0, 64-127 head1
  scores per k-block: two row-tiled matmuls (h0 rows 0-63, h1 rows 64-127
  of the PE array, concurrent) -> one [128 k, 1024] PSUM tile (h0|h1)
  exp via ScalarE (scale=1/8 folded), causal mask multiply only on
  diagonal blocks; ex bf16.
  attnT accumulated as [65, 512] fp32 per head: rows 0-63 v^T ex, row 64
  softmax denominators (ones column appended to v).
  normalize: reciprocal of denominators + ones-outer-product broadcast
  matmul, fused into the PSUM->SBUF evacuation multiply.
"""

import numpy as np

B, S, D, H = 2, 4096, 512, 8
DH = 64
QCHUNK = 512
SCALE = 1.0 / np.sqrt(DH)

_CACHE = {}


def _build(s=S, normalize=True, repeats=1):
    from contextlib import ExitStack

    import concourse.mybir as mybir
    import concourse.tile as tile
    from concourse import bacc

    f32 = mybir.dt.float32
    f32r = mybir.dt.float32r
    bf16 = mybir.dt.bfloat16
    EXP = mybir.ActivationFunctionType.Exp
    GE = mybir.AluOpType.is_ge

    nqc = s // QCHUNK      # q chunks
    nkb_all = s // 128     # k blocks
    ndc = D // 128         # D chunks (contraction for projections)
    kb_per_chunk = QCHUNK // 128

    nc = bacc.Bacc("TRN2")
    xT_d = nc.dram_tensor("xT", [D, s], bf16, kind="ExternalInput")
    wqT_d = nc.dram_tensor("wqT", [D, 128], bf16, kind="ExternalInput")
    wkT_d = nc.dram_tensor("wkT", [D, 128], bf16, kind="ExternalInput")
    wvT_d = nc.dram_tensor("wvT", [D, 128], bf16, kind="ExternalInput")
    wpT_d = nc.dram_tensor("wpT", [128, D], bf16, kind="ExternalInput")
    ones_d = nc.dram_tensor("ones_in", [128, 64], bf16, kind="ExternalInput")
    out_d = nc.dram_tensor("out_p", [s, D], f32, kind="ExternalOutput")

    with ExitStack() as ctx:
        tc = ctx.enter_context(tile.TileContext(nc))
        consts = ctx.enter_context(tc.tile_pool(name="consts", bufs=1))
        big = ctx.enter_context(tc.tile_pool(name="big", bufs=1))
        expool = ctx.enter_context(tc.tile_pool(name="expool", bufs=4))
        denpool = ctx.enter_context(tc.tile_pool(name="denpool", bufs=2))
        outpool = ctx.enter_context(tc.tile_pool(name="outpool", bufs=2))
        mmps = ctx.enter_context(tc.tile_pool(name="mmps", bufs=2, space="PSUM"))
        scps = ctx.enter_context(tc.tile_pool(name="scps", bufs=2, space="PSUM"))
        accps = ctx.enter_context(tc.tile_pool(name="accps", bufs=1, space="PSUM"))

        # ---- persistent SBUF ----
        xT = [big.tile([128, s], bf16, name=f"xT{c}", tag=f"xT{c}") for c in range(ndc)]
        qT = big.tile([128, s], bf16, name="qT", tag="qT")
        kT = big.tile([128, s], bf16, name="kT", tag="kT")
        v65 = [big.tile([128, 65 * nkb_all], bf16, name=f"v65_{h}", tag=f"v65_{h}")
               for h in range(2)]
        attnT = big.tile([128, s], bf16, name="attnT", tag="attnT")
        wq = consts.tile([128, D], bf16, name="wq", tag="wq")
        wk = consts.tile([128, D], bf16, name="wk", tag="wk")
        wv = consts.tile([128, D], bf16, name="wv", tag="wv")
        wp = consts.tile([128, D], bf16, name="wp", tag="wp")
        masks = [consts.tile([128, 1024], bf16, name=f"mask{r}", tag=f"mask{r}")
                 for r in range(kb_per_chunk)]
        ones33 = consts.tile([33, 64], f32r, name="ones33", tag="ones33")

        for _rep in range(repeats):
            _emit_body(nc, tc, locals())

    nc.compile()
    return nc


def _emit_body(nc, tc, env):
    """One full pass of the kernel body (DMAs + all chunks)."""
    import concourse.mybir as mybir

    f32 = mybir.dt.float32
    f32r = mybir.dt.float32r
    bf16 = mybir.dt.bfloat16
    EXP = mybir.ActivationFunctionType.Exp
    GE = mybir.AluOpType.is_ge
    (s, nqc, nkb_all, ndc, kb_per_chunk, normalize) = (
        env["s"], env["nqc"], env["nkb_all"], env["ndc"],
        env["kb_per_chunk"], env["normalize"])
    (xT_d, wqT_d, wkT_d, wvT_d, wpT_d, ones_d, out_d) = (
        env["xT_d"], env["wqT_d"], env["wkT_d"], env["wvT_d"], env["wpT_d"],
        env["ones_d"], env["out_d"])
    (xT, qT, kT, v65, attnT, wq, wk, wv, wp, masks, ones33) = (
        env["xT"], env["qT"], env["kT"], env["v65"], env["attnT"], env["wq"],
        env["wk"], env["wv"], env["wp"], env["masks"], env["ones33"])
    (consts, big, expool, denpool, outpool, mmps, scps, accps) = (
        env["consts"], env["big"], env["expool"], env["denpool"],
        env["outpool"], env["mmps"], env["scps"], env["accps"])
    QCHUNK = 512

    # ---- input DMAs (weights + first x slice first: fastest path to the
    #      first score matmul / exp) ----
    half = s // 2
    for w_sb, w_d in ((wq, wqT_d), (wk, wkT_d), (wv, wvT_d)):
        nc.sync.dma_start(
            out=w_sb.rearrange("p (c j) -> p c j", j=128),
            in_=w_d.rearrange("(c p) j -> p c j", p=128))
    for lo, hi in ((0, QCHUNK), (QCHUNK, half), (half, s)):
        for c in range(ndc):
            nc.sync.dma_start(out=xT[c][:, lo:hi],
                              in_=xT_d[c * 128:(c + 1) * 128, lo:hi])
    nc.sync.dma_start(out=wp, in_=wpT_d.ap())
    for h in range(2):
        ones_ap = v65[h].rearrange("p (k c) -> p k c", c=65)[:, :, 64]
        nc.sync.dma_start(out=ones_ap, in_=ones_d[:, 0:nkb_all])
    nc.gpsimd.memset(ones33.bitcast(f32), 1.0)

    # ---- causal masks: mask[r][p, f%512] = 1.0 where (f%512) >= p + 128*r ----
    mtmp = denpool.tile([128, 512], f32, name="mtmp", tag="mtmp")
    for r in range(kb_per_chunk):
        nc.gpsimd.memset(mtmp, 1.0)
        nc.gpsimd.affine_select(
            out=mtmp, in_=mtmp, compare_op=GE, fill=0.0,
            base=-128 * r, channel_multiplier=-1, pattern=[[1, 512]])
        nc.gpsimd.tensor_copy(masks[r][:, 0:512], mtmp)
        nc.gpsimd.tensor_copy(masks[r][:, 512:1024], mtmp)

    def phase1_units(qc):
        """Projection work for q-range qc as independently-emittable units
        (interleaved into the previous chunk's k-block loop so the PE fills
        its exp-wait stalls with them)."""
        qs = slice(qc * QCHUNK, (qc + 1) * QCHUNK)

        def proj_unit(w_sb, dst):
            ps = mmps.tile([128, QCHUNK], f32, name=f"proj_{qc}", tag="mm")
            for c in range(ndc):
                nc.tensor.matmul(ps,
                                 lhsT=w_sb[:, c * 128:(c + 1) * 128],
                                 rhs=xT[c][:, qs],
                                 start=(c == 0), stop=(c == ndc - 1))
            nc.vector.tensor_copy(dst[:, qs], ps)

        def v_unit(j):
            # natural-layout v rows: one [128,128] block (both heads)
            kb = qc * kb_per_chunk + j
            vp = mmps.tile([128, 128], f32, name=f"vp_{kb}", tag="mm")
            for c in range(ndc):
                nc.tensor.matmul(vp,
                                 lhsT=xT[c][:, kb * 128:(kb + 1) * 128],
                                 rhs=wv[:, c * 128:(c + 1) * 128],
                                 start=(c == 0), stop=(c == ndc - 1))
            for h in range(2):
                nc.vector.tensor_copy(v65[h][:, kb * 65:kb * 65 + 64],
                                      vp[:, h * 64:(h + 1) * 64])

        units = [lambda w=w, d=d: proj_unit(w, d)
                 for w, d in ((wq, qT), (wk, kT))]
        units += [lambda j=j: v_unit(j) for j in range(kb_per_chunk)]
        return units

    for u in phase1_units(0):
        u()
    for qc in range(nqc):
        qlo = qc * QCHUNK
        qs = slice(qlo, qlo + QCHUNK)

        # ---- phase 2: attention over k blocks, next chunk's projections
        #      interleaved ----
        nkb = (qc + 1) * kb_per_chunk
        units = phase1_units(qc + 1) if qc + 1 < nqc else []
        nunits, emitted_units = len(units), 0
        acc = [accps.tile([65, QCHUNK], f32, name=f"acc{h}_{qc}", tag=f"acc{h}")
               for h in range(2)]
        for kb in range(nkb):
            sc = scps.tile([128, 1024], f32, name=f"sc_{qc}_{kb}", tag="sc")
            for h in range(2):
                hsl = slice(h * 64, (h + 1) * 64)
                nc.tensor.matmul(
                    sc[:, h * 512:(h + 1) * 512],
                    lhsT=kT[hsl, kb * 128:(kb + 1) * 128],
                    rhs=qT[hsl, qs],
                    start=True, stop=True)
            ex = expool.tile([128, 1024], bf16, name=f"ex_{qc}_{kb}", tag="ex")
            nc.scalar.activation(ex, sc, EXP, scale=float(SCALE))
            if kb * 128 >= qlo:  # diagonal block: zero where k > q
                # gpsimd (otherwise idle) so DVE stays free for evacuations
                nc.gpsimd.tensor_mul(ex, ex, masks[kb - qc * kb_per_chunk])
            for h in range(2):
                nc.tensor.matmul(
                    acc[h],
                    lhsT=v65[h][:, kb * 65:(kb + 1) * 65],
                    rhs=ex[:, h * 512:(h + 1) * 512],
                    start=(kb == 0), stop=(kb == nkb - 1))
            # spread next-chunk projection units across this kb loop
            while units and emitted_units * nkb <= (kb + 1) * nunits:
                units.pop(0)()
                emitted_units += 1

        for u in units:
            u()

        # ---- phase 3: normalize + output projection for this q-chunk ----
        if normalize:
            # head h's denominator row sits at partition 32*h (32-aligned)
            dens = denpool.tile([33, QCHUNK], f32r, name=f"dens_{qc}",
                                tag="dens")
            for h in range(2):
                nc.vector.tensor_copy(dens[32 * h:32 * h + 1, :],
                                      acc[h][64:65, :])
            for h in range(2):
                hsl = slice(h * 64, (h + 1) * 64)
                # broadcast den across 64 partitions via ones outer-product,
                # then approx-reciprocal evacuates PSUM->SBUF
                bc = mmps.tile([64, QCHUNK], f32, name=f"bc{h}_{qc}", tag="mm")
                nc.tensor.matmul(bc, lhsT=ones33[32 * h:32 * h + 1, 0:64],
                                 rhs=dens[32 * h:32 * h + 1, :],
                                 start=True, stop=True)
                rbc = denpool.tile([64, QCHUNK], f32, name=f"rbc{h}_{qc}",
                                   tag=f"rbc{h}")
                nc.vector.reciprocal_approx_fast(out=rbc, in_=bc)
                nc.vector.tensor_mul(attnT[hsl, qs], acc[h][0:64, :], rbc)
        else:
            for h in range(2):
                nc.vector.tensor_copy(attnT[h * 64:(h + 1) * 64, qs],
                                      acc[h][0:64, :])

        ob = outpool.tile([128, kb_per_chunk * D], f32, name=f"ob_{qc}",
                          tag="ob")
        for j in range(kb_per_chunk):
            qb = qc * kb_per_chunk + j
            pp = mmps.tile([128, D], f32, name=f"pp_{qb}", tag="mm")
            nc.tensor.matmul(pp,
                             lhsT=attnT[:, qb * 128:(qb + 1) * 128],
                             rhs=wp, start=True, stop=True)
            nc.vector.tensor_copy(ob[:, j * D:(j + 1) * D], pp)
        nc.sync.dma_start(
            out=out_d.rearrange("(q j p) d -> q p j d", j=kb_per_chunk,
                                p=128)[qc],
            in_=ob.rearrange("p (j d) -> p j d", d=D))


def _in_maps(x, W_q, W_k, W_v, W_proj):
    import ml_dtypes
    bf16 = ml_dtypes.bfloat16
    maps = []
    for c in range(8):
        b, hp = c // 4, c % 4
        cols = slice(hp * 128, (hp + 1) * 128)
        maps.append({
            "xT": np.ascontiguousarray(x[b].T).astype(bf16),
            "wqT": np.ascontiguousarray(W_q.T[:, cols]).astype(bf16),
            "wkT": np.ascontiguousarray(W_k.T[:, cols]).astype(bf16),
            "wvT": np.ascontiguousarray(W_v.T[:, cols]).astype(bf16),
            "wpT": np.ascontiguousarray(W_proj[:, cols].T).astype(bf16),
            "ones_in": np.ones((128, 64), dtype=bf16),
        })
    return maps


def kernel(x, W_q, W_k, W_v, W_proj, b_proj, _trace=False):
    from concourse.bass_utils import run_bass_kernel_spmd

    x = np.asarray(x, dtype=np.float32)
    W_q = np.asarray(W_q, dtype=np.float32)
    W_k = np.asarray(W_k, dtype=np.float32)
    W_v = np.asarray(W_v, dtype=np.float32)
    W_proj = np.asarray(W_proj, dtype=np.float32)
    b_proj = np.asarray(b_proj, dtype=np.float32)

    if "nc" not in _CACHE:
        _CACHE["nc"] = _build()
    nc = _CACHE["nc"]

    res = run_bass_kernel_spmd(nc, _in_maps(x, W_q, W_k, W_v, W_proj),
                               core_ids=list(range(8)), trace=_trace)
    out = np.empty((B, S, D), dtype=np.float32)
    for b in range(B):
        acc = res.results[4 * b]["out_p"].astype(np.float32)
        for j in range(1, 4):
            acc = acc + res.results[4 * b + j]["out_p"]
        out[b] = acc + b_proj
    if _trace:
        _CACHE["last_trace"] = res
    return out
